# revision 10
# baseline (speedup 1.0000x reference)
"""Contrastive cross-lingual loss kernel for Trainium2 (8 NeuronCores).

Reference math (B=128, S=2048, H=1024, C=256):
    pooled = (features * mask).sum(S) / mask.sum(S)          # [B,H]
    h      = relu(pooled @ W1 + b1)                          # [B,H]
    proj   = h @ W2 + b2                                     # [B,C]
    proj   = proj / max(||proj||, 1e-12)                     # [B,C]
    sim    = proj @ proj.T / temperature                     # [B,B]
    labels = lang[i] != lang[j]                              # off-diag handled implicitly
    loss   = mean_i( -sum_j labels * log_softmax(sim)[i,j] )

Sharding: data-parallel on batch (16 rows/core).  Each core streams its
128 MiB feature shard once; masked mean-pool runs on the TensorEngine with
the feature tile as the stationary operand (fp32-exact) and the mask column
as the moving operand, producing pooledT [h, b] directly in PSUM.  The MLP
head + L2-norm run per-core, the [16,256] projections are all-gathered
across the 8 cores, and each core computes its 16 rows of the contrastive
loss.  The host concatenates projections and means the 128 row losses.
"""

import sys

if "/opt/trn_rl_repo" not in sys.path:
    sys.path.insert(0, "/opt/trn_rl_repo")

from contextlib import ExitStack

import numpy as np

import concourse.bass as bass
import concourse.bacc as bacc
import concourse.mybir as mybir
import concourse.tile as tile
from concourse.masks import make_identity

F32 = mybir.dt.float32
I32 = mybir.dt.int32

B, S, H, C = 128, 2048, 1024, 256
NCORES = 8


def _pbcast(src: bass.AP, n: int) -> bass.AP:
    """Broadcast a DRAM AP whose first dim is the 'row' dim across n
    partitions: [r, 1] -> [n, r] (step-0 partition dim)."""
    a = list(src.ap)
    return bass.AP(tensor=src.tensor, offset=src.offset, ap=[[0, n], list(a[0])])


def build_program(b=B, s=S, h=H, c=C, ncores=NCORES, feat_bufs=3):
    BL = b // ncores          # local batch rows
    ST = s // 128             # s-tiles per row
    G = 2 if ST % 2 == 0 else 1   # feature DMA granularity: half a batch row
    SG = ST // G
    HC = h // 128             # h chunks of 128
    CC = c // 128             # c chunks of 128
    N1 = min(512, h)          # psum free-dim chunk for W1 matmuls
    NCH = h // N1

    nc = bacc.Bacc("TRN2", num_devices=ncores)

    feat = nc.dram_tensor("feat", [BL, s, h], F32, kind="ExternalInput")
    maskt = nc.dram_tensor("maskt", [128, ST, BL], F32, kind="ExternalInput")
    maski = nc.dram_tensor("maski", [BL, s], I32, kind="ExternalInput")
    w1 = nc.dram_tensor("w1", [h, h], F32, kind="ExternalInput")
    b1 = nc.dram_tensor("b1", [1, h], F32, kind="ExternalInput")
    w2 = nc.dram_tensor("w2", [h, c], F32, kind="ExternalInput")
    b2 = nc.dram_tensor("b2", [1, c], F32, kind="ExternalInput")
    tempr = nc.dram_tensor("tempr", [1, 1], F32, kind="ExternalInput")
    langrow = nc.dram_tensor("langrow", [BL, b], F32, kind="ExternalInput")
    langloc = nc.dram_tensor("langloc", [BL, 1], F32, kind="ExternalInput")

    out_proj = nc.dram_tensor("out_proj", [BL, c], F32, kind="ExternalOutput")
    out_rloss = nc.dram_tensor("out_rloss", [BL, 1], F32, kind="ExternalOutput")

    AF = mybir.ActivationFunctionType
    ALU = mybir.AluOpType

    with tile.TileContext(nc) as tc, ExitStack() as ctx:
        fpool = ctx.enter_context(tc.tile_pool(name="fpool", bufs=feat_bufs))
        wpool = ctx.enter_context(tc.tile_pool(name="wpool", bufs=1))
        small = ctx.enter_context(tc.tile_pool(name="small", bufs=1))
        work = ctx.enter_context(tc.tile_pool(name="work", bufs=1))
        ppool = ctx.enter_context(tc.tile_pool(name="ppool", bufs=1, space="PSUM"))
        ptile = ctx.enter_context(tc.tile_pool(name="ptile", bufs=2, space="PSUM"))
        pmm = ctx.enter_context(tc.tile_pool(name="pmm", bufs=1, space="PSUM"))
        dpool = ctx.enter_context(tc.tile_pool(name="dram", bufs=1, space="DRAM"))

        # ---- constants / small loads -------------------------------------
        maskt_sb = small.tile([128, ST, BL], F32)
        nc.sync.dma_start(out=maskt_sb, in_=maskt[:])
        mask_sb = small.tile([BL, s], F32)
        nc.gpsimd.dma_start(out=mask_sb, in_=maski[:])  # i32 -> f32 cast DMA
        w1_sb = wpool.tile([128, HC, h], F32)
        nc.sync.dma_start(out=w1_sb, in_=w1[:].rearrange("(kc p) n -> p kc n", p=128))
        w2_sb = wpool.tile([128, HC, c], F32)
        nc.sync.dma_start(out=w2_sb, in_=w2[:].rearrange("(kc p) n -> p kc n", p=128))
        b1_sb = small.tile([1, h], F32)
        nc.sync.dma_start(out=b1_sb, in_=b1[:])
        b2_sb = small.tile([1, c], F32)
        nc.sync.dma_start(out=b2_sb, in_=b2[:])
        lrow_sb = small.tile([BL, b], F32)
        nc.sync.dma_start(out=lrow_sb, in_=langrow[:])
        lloc_sb = small.tile([BL, 1], F32)
        nc.sync.dma_start(out=lloc_sb, in_=langloc[:])
        t11 = small.tile([1, 1], F32)
        nc.sync.dma_start(out=t11, in_=tempr[:])
        ident = small.tile([128, 128], F32)
        make_identity(nc, ident)
        ones_r = small.tile([1, 128], F32)
        nc.vector.memset(ones_r, 1.0)
        ones_sb = ones_r[:, :BL]

        # temperature broadcast [1,1] -> [BL,1] via ones matmul, then 1/t
        tb_ps = ptile.tile([BL, 1], F32, tag="bc", bufs=1)
        nc.tensor.matmul(tb_ps, lhsT=ones_sb, rhs=t11, start=True, stop=True)
        t16 = small.tile([BL, 1], F32)
        nc.vector.tensor_copy(out=t16, in_=tb_ps)

        # ---- mask count -> 1/count, broadcast across partitions ----------
        cnt = small.tile([BL, 1], F32)
        nc.vector.reduce_sum(out=cnt, in_=mask_sb, axis=mybir.AxisListType.X)
        invc = small.tile([BL, 1], F32)
        nc.vector.reciprocal(invc, cnt)
        # transpose to a row, then broadcast down 128 partitions via ones
        icr_ps = ptile.tile([1, BL], F32, tag="bc", bufs=1)
        nc.tensor.transpose(icr_ps, invc, ident[:BL, :BL])
        icr = small.tile([1, BL], F32)
        nc.vector.tensor_copy(out=icr, in_=icr_ps)
        invb_ps = ptile.tile([128, BL], F32, tag="bc", bufs=1)
        nc.tensor.matmul(invb_ps, lhsT=ones_r, rhs=icr, start=True, stop=True)
        invb = small.tile([128, BL], F32)
        nc.vector.tensor_copy(out=invb, in_=invb_ps)

        # ---- masked-sum pooling: pooledT[h, b, g] partial sums in PSUM ---
        # fview[b, g, p, sg, h]  <->  feat[b, (g*SG+sg)*128+p, h]
        fview = feat[:].rearrange("b (g sg p) h -> b g p sg h", g=G, sg=SG, p=128)
        pooled_ps = ppool.tile([128, HC, BL, G], F32)
        for bi in range(BL):
            for g in range(G):
                ft = fpool.tile([128, SG, h], F32, tag="ft")
                nc.sync.dma_start(out=ft, in_=fview[bi, g])
                for hc in range(HC):
                    for sg in range(SG):
                        nc.tensor.matmul(
                            pooled_ps[:, hc, bi, g : g + 1],
                            lhsT=ft[:, sg, hc * 128 : (hc + 1) * 128],
                            rhs=maskt_sb[:, g * SG + sg, bi : bi + 1],
                            start=(sg == 0),
                            stop=(sg == SG - 1),
                        )

        # pooled mean (transposed layout): pooledT = sum_g(pooled_ps) / count[b]
        pooledT_sb = work.tile([128, HC, BL], F32)
        for hc in range(HC):
            if G > 1:
                # sum the G partial columns (single PSUM-read instruction)
                psum_b = work.tile([128, BL], F32, tag="psum_b")
                nc.vector.reduce_sum(
                    out=psum_b, in_=pooled_ps[:, hc, :, :],
                    axis=mybir.AxisListType.X,
                )
            else:
                psum_b = pooled_ps[:, hc, :, 0]
            nc.vector.tensor_mul(
                out=pooledT_sb[:, hc, :], in0=psum_b, in1=invb
            )

        # ---- MLP layer 1: h = relu(pooled @ W1 + b1) ---------------------
        h_sb = work.tile([BL, h], F32)
        for nch in range(NCH):
            ph = pmm.tile([BL, N1], F32, tag="ph")
            nsl = slice(nch * N1, (nch + 1) * N1)
            for kc in range(HC):
                nc.tensor.matmul(
                    ph,
                    lhsT=pooledT_sb[:, kc, :],
                    rhs=w1_sb[:, kc, nsl],
                    start=(kc == 0),
                    stop=False,
                )
            nc.tensor.matmul(
                ph, lhsT=ones_sb, rhs=b1_sb[:, nsl], start=False, stop=True
            )
            nc.scalar.activation(out=h_sb[:, nsl], in_=ph, func=AF.Relu)

        # transpose h -> hT [h, b]
        hT_sb = work.tile([128, HC, BL], F32)
        for hc in range(HC):
            pt = ptile.tile([128, BL], F32, tag="pt")
            nc.tensor.transpose(
                pt, h_sb[:, hc * 128 : (hc + 1) * 128], ident[:BL, :BL]
            )
            nc.vector.tensor_copy(out=hT_sb[:, hc, :], in_=pt)

        # ---- MLP layer 2: proj = h @ W2 + b2 -----------------------------
        pp = pmm.tile([BL, c], F32, tag="pp")
        for kc in range(HC):
            nc.tensor.matmul(
                pp, lhsT=hT_sb[:, kc, :], rhs=w2_sb[:, kc, :],
                start=(kc == 0), stop=False,
            )
        nc.tensor.matmul(pp, lhsT=ones_sb, rhs=b2_sb[:], start=False, stop=True)

        # ---- L2 normalize ------------------------------------------------
        proj_sb = work.tile([BL, c], F32)
        nc.vector.tensor_copy(out=proj_sb, in_=pp)
        sq_sb = work.tile([BL, c], F32)
        ss = small.tile([BL, 1], F32)
        nc.scalar.activation(out=sq_sb, in_=pp, func=AF.Square, accum_out=ss)
        nrm = small.tile([BL, 1], F32)
        nc.scalar.activation(out=nrm, in_=ss, func=AF.Sqrt)
        nc.vector.tensor_scalar_max(out=nrm, in0=nrm, scalar1=1e-12)
        invn = small.tile([BL, 1], F32)
        nc.vector.reciprocal(invn, nrm)
        projn_sb = work.tile([BL, c], F32)
        nc.vector.tensor_scalar_mul(out=projn_sb, in0=proj_sb, scalar1=invn)

        # ---- all-gather projections over the 8 cores ---------------------
        ag_in = dpool.tile([BL, c], F32)
        nc.sync.dma_start(out=ag_in, in_=projn_sb)
        ag_out = dpool.tile([b, c], F32, addr_space="Shared")
        nc.gpsimd.collective_compute(
            "AllGather",
            ALU.bypass,
            replica_groups=[list(range(ncores))],
            ins=[ag_in.opt()],
            outs=[ag_out.opt()],
        )
        pall_sb = work.tile([b, c], F32)
        nc.sync.dma_start(out=pall_sb, in_=ag_out[:])

        # ---- similarity rows: sim[local, global] -------------------------
        pT_sb = work.tile([128, CC, BL], F32)
        for cc in range(CC):
            pt2 = ptile.tile([128, BL], F32, tag="pt")
            nc.tensor.transpose(
                pt2, projn_sb[:, cc * 128 : (cc + 1) * 128], ident[:BL, :BL]
            )
            nc.vector.tensor_copy(out=pT_sb[:, cc, :], in_=pt2)
        pallT_sb = work.tile([128, CC, b], F32)
        for cc in range(CC):
            pt3 = ptile.tile([128, b], F32, tag="pt3", bufs=1)
            nc.tensor.transpose(pt3, pall_sb[:, cc * 128 : (cc + 1) * 128],
                                ident[:b, :b])
            nc.vector.tensor_copy(out=pallT_sb[:, cc, :], in_=pt3)

        ps_sim = pmm.tile([BL, b], F32, tag="psim")
        for cc in range(CC):
            nc.tensor.matmul(
                ps_sim, lhsT=pT_sb[:, cc, :], rhs=pallT_sb[:, cc, :],
                start=(cc == 0), stop=(cc == CC - 1),
            )
        invt = small.tile([BL, 1], F32)
        nc.vector.reciprocal(invt, t16)
        lg_sb = work.tile([BL, b], F32)
        nc.vector.tensor_scalar_mul(out=lg_sb, in0=ps_sim, scalar1=invt)

        # ---- row losses: nlab*(max+lse) - sum(labels*logits) -------------
        nmx = small.tile([BL, 1], F32)
        nc.vector.reduce_max(out=nmx, in_=lg_sb, axis=mybir.AxisListType.X,
                             negate=True)
        e_sb = work.tile([BL, b], F32)
        se = small.tile([BL, 1], F32)
        nc.scalar.activation(out=e_sb, in_=lg_sb, func=AF.Exp, bias=nmx,
                             scale=1.0, accum_out=se)
        ls = small.tile([BL, 1], F32)
        nc.scalar.activation(out=ls, in_=se, func=AF.Ln)

        lab_sb = work.tile([BL, b], F32)
        nc.vector.tensor_scalar(
            out=lab_sb, in0=lrow_sb, scalar1=lloc_sb, scalar2=None,
            op0=ALU.not_equal,
        )
        tt_sb = work.tile([BL, b], F32)
        s1 = small.tile([BL, 1], F32)
        nc.vector.tensor_mul(out=tt_sb, in0=lab_sb, in1=lg_sb)
        nc.vector.reduce_sum(out=s1, in_=tt_sb, axis=mybir.AxisListType.X)
        nlab = small.tile([BL, 1], F32)
        nc.vector.reduce_sum(out=nlab, in_=lab_sb, axis=mybir.AxisListType.X)
        mpls = small.tile([BL, 1], F32)
        nc.vector.tensor_sub(out=mpls, in0=ls, in1=nmx)  # ls + max
        t1s = small.tile([BL, 1], F32)
        nc.vector.tensor_mul(out=t1s, in0=mpls, in1=nlab)
        rl = small.tile([BL, 1], F32)
        nc.vector.tensor_sub(out=rl, in0=t1s, in1=s1)

        nc.sync.dma_start(out=out_proj[:], in_=projn_sb)
        nc.sync.dma_start(out=out_rloss[:], in_=rl)

    nc.compile()
    return nc


def make_in_maps(features, W1, b1, W2, b2, temperature, language_ids,
                 attention_mask, ncores=NCORES):
    """Shard full inputs into per-core input maps."""
    b, s, h = features.shape
    c = W2.shape[1]
    BL = b // ncores
    ST = s // 128
    lang_f = np.ascontiguousarray(language_ids.astype(np.float32))
    lrow = np.ascontiguousarray(
        np.broadcast_to(lang_f[None, :], (BL, b)).astype(np.float32)
    )
    w1_in = np.ascontiguousarray(W1.astype(np.float32))
    b1_in = np.ascontiguousarray(b1.astype(np.float32).reshape(1, h))
    w2_in = np.ascontiguousarray(W2.astype(np.float32))
    b2_in = np.ascontiguousarray(b2.astype(np.float32).reshape(1, c))
    t_in = np.ascontiguousarray(temperature.astype(np.float32).reshape(1, 1))

    in_maps = []
    for ci in range(ncores):
        rows = slice(ci * BL, (ci + 1) * BL)
        mask_c = np.ascontiguousarray(attention_mask[rows].astype(np.int32))
        # maskt[p, st, b] = mask[b, st*128 + p]
        maskt_c = np.ascontiguousarray(
            mask_c.reshape(BL, ST, 128).transpose(2, 1, 0).astype(np.float32)
        )
        in_maps.append({
            "feat": np.ascontiguousarray(features[rows].astype(np.float32)),
            "maskt": maskt_c,
            "maski": mask_c,
            "w1": w1_in,
            "b1": b1_in,
            "w2": w2_in,
            "b2": b2_in,
            "tempr": t_in,
            "langrow": lrow,
            "langloc": np.ascontiguousarray(lang_f[rows].reshape(BL, 1)),
        })
    return in_maps


_PROGRAM_CACHE = {}


def _get_program():
    if "nc" not in _PROGRAM_CACHE:
        _PROGRAM_CACHE["nc"] = build_program()
    return _PROGRAM_CACHE["nc"]


def kernel(features, W1, b1, W2, b2, temperature, language_ids,
           attention_mask, _trace=False):
    from concourse.bass_utils import run_bass_kernel_spmd

    nc = _get_program()
    in_maps = make_in_maps(
        np.asarray(features), np.asarray(W1), np.asarray(b1), np.asarray(W2),
        np.asarray(b2), np.asarray(temperature), np.asarray(language_ids),
        np.asarray(attention_mask),
    )
    res = run_bass_kernel_spmd(
        nc, in_maps, core_ids=list(range(NCORES)), trace=_trace
    )
    proj = np.concatenate([r["out_proj"] for r in res.results], axis=0)
    rloss = np.concatenate([r["out_rloss"] for r in res.results], axis=0)
    loss = np.float32(rloss.astype(np.float64).sum() / B)
    if _trace:
        kernel.last_results = res
    return proj, loss
